# revision 58
# baseline (speedup 1.0000x reference)
"""MoE router kernel for Trainium2 (Bass/Tile), 8-core data-parallel SPMD.

Per row b (B=4096 total, 512 per core, laid out as 128 partitions x 4 groups):
  expert_logits[b,e] = sum_f x[b,e,f]*v_e[f] + x_ctx[b].v_c + const   (E=64, f=7)
  gate_weights[b,:]  = softmax over top-8 of expert_logits (others 0)
  action_logits[b,a] = sum_e gate[b,e] * x_q[b,e,a]                    (a=3)
with v_e = We @ Ws[:H], v_c = Wc @ Ws[H:], const = bc.Ws[H:] + be.Ws[:H] + bs
folded on device by tiny PE matmuls over host-transposed weights, then
replicated across partitions with a ones-matmul. (Associativity fold
validated: top-8 sets match the reference exactly on the fixed seed inputs,
and logits are bounded |l|<3 so exp() without max-subtraction is safe in f32.)

Inputs are host-marshalled (pure layout, no host math): per-expert features
reordered feature-major so all DVE reads are unit-stride, weights stacked and
transposed into one [512, 81] matrix = [ (Wc|bc|We|be|bs-row)^T | Ws_c | Ws_e | e0 ].
"""

import sys

for _p in ("/opt/trn_rl_repo", "/root/.axon_site/_ro/trn_rl_repo"):
    if _p not in sys.path:
        sys.path.append(_p)

import numpy as np

import concourse.bass as bass
import concourse.mybir as mybir
from concourse.bass_types import AP
from concourse.bass_utils import run_bass_kernel_spmd
from concourse.tile import TileContext

F32 = mybir.dt.float32
ALU = mybir.AluOpType
ACTF = mybir.ActivationFunctionType

B, E, H, TOPK = 4096, 64, 512, 8
NCORES = 8
BS = B // NCORES          # rows per core = 512
P = 128                   # partitions
G = BS // P               # row groups per core = 4
FQ, FR, FK = 3, 2, 2      # per-expert feature widths
NF = FQ + FR + FK         # 7
NW = 68 + 1 + NF + 1 + 1  # stacked rows: Wc | bc | We | be | bs-row = 78
KC = H // P               # 4 contraction chunks


def _bview(ap: AP, free_dims):
    """View `ap` with explicit free-dim (step, count) pairs (partition dim kept)."""
    return AP(ap.tensor, ap.offset, [list(ap.ap[0])] + [list(d) for d in free_dims])


def _split_waits(nc: bass.Bass) -> None:
    """walrus codegen in this environment supports a single sync-wait slot per
    instruction; peel extra waits onto cloned harmless same-engine ops placed
    just before (Drains are peeled onto extra Drains)."""
    import copy

    templates = nc._carrier_templates
    n = 0
    for f in nc.m.functions:
        for blk in f.blocks:
            out = []
            for ins in blk.instructions:
                si = ins.sync_info
                tmpl = templates.get(getattr(ins, "engine", None))
                if type(ins).__name__ == "InstISA" and tmpl is not None:
                    # walrus rejects the barrier's engine-nop ISA op; swap in a
                    # harmless same-engine memset with identical sync_info
                    car = copy.deepcopy(tmpl)
                    car.name = ins.name
                    car.sync_info = si
                    out.append(car)
                    continue
                is_drain = type(ins).__name__ == "InstDrain"
                if is_drain and si is not None and si.on_wait:
                    # the all-engine barrier that follows already guarantees
                    # engine completion; the drain only needs DMA-queue quiesce
                    dma_waits = [w for w in si.on_wait if (w.ant_name or "").startswith("DMA")]
                    if dma_waits:
                        si.on_wait = dma_waits
                        ins.sync_info = si
                if si is not None and si.on_wait and len(si.on_wait) > 1 and (tmpl is not None or is_drain):
                    waits = list(si.on_wait)
                    for w in waits[:-1]:
                        if is_drain:
                            car = mybir.InstDrain(name=f"waitcar-{n}", ins=[], outs=[])
                            car.engine = ins.engine
                        else:
                            car = copy.deepcopy(tmpl)
                            car.name = f"waitcar-{n}"
                        n += 1
                        car.sync_info = mybir.SyncInfo(on_wait=[w], on_update=[])
                        out.append(car)
                    si.on_wait = [waits[-1]]
                    ins.sync_info = si
                out.append(ins)
            blk.instructions = out


def _strip_tail(nc: bass.Bass) -> None:
    """Drop the startup all-engine barrier and the end-of-kernel barriers +
    per-semaphore clears. They exist so a loaded NEFF can be re-executed;
    every kernel() call here builds a fresh executable (semaphores are reset
    at NEFF load), and nothing in this kernel depends on the prologue const-AP
    memsets (Exp uses an explicit zero-bias tile). Only the DMA-quiesce drain
    (output data reaching DRAM) must stay."""
    for f in nc.m.functions:
        for blk in f.blocks:
            insts = list(blk.instructions)
            if not insts:
                continue
            if blk.name == "main":
                blk.instructions = [
                    i for i in insts
                    if type(i).__name__ not in ("InstDrain", "InstEventSemaphore")
                ]
                continue
            first = insts[0]
            if type(first).__name__ != "InstDrain":
                continue
            si = first.sync_info
            if not (si and si.on_wait and any((w.ant_name or "").startswith("DMA") for w in si.on_wait)):
                continue
            kept = [first]
            for ins in insts[1:]:
                if type(ins).__name__ in ("InstEventSemaphore", "InstDrain", "InstISA"):
                    continue
                kept.append(ins)
            blk.instructions = kept


def _build_program() -> bass.Bass:
    nc = bass.Bass()

    # per-expert features, feature-major: [b, f*64+e]; f: q0 q1 q2 r0 r1 k0 k1
    xin_d = nc.declare_dram_parameter("xin", [BS, NF * E], F32, isOutput=False)
    xc_d = nc.declare_dram_parameter("xc", [BS, 68], F32, isOutput=False)
    # [ WX^T (78 cols) | Ws_c | Ws_e | e0 ]; WX rows = Wc|bc|We|be|bs-row.
    # Two halves on separate DMA queues so the fold starts after half A lands.
    wta_d = nc.declare_dram_parameter("WTA", [H // 2, NW + 3], F32, isOutput=False)
    wtb_d = nc.declare_dram_parameter("WTB", [H // 2, NW + 3], F32, isOutput=False)

    # outputs: [expert_logits(64) | gates(64)] per row, and [action(3) | pad]
    ego_d = nc.declare_dram_parameter("ego", [BS, 2 * E], F32, isOutput=True)
    act_d = nc.declare_dram_parameter("acto", [BS, FQ + 1], F32, isOutput=True)

    with TileContext(nc) as tc:
        with (
            tc.tile_pool(name="sb", bufs=1) as sb,
            tc.tile_pool(name="ps", bufs=1, space="PSUM") as ps,
        ):
            # ---------------- weight folding on PE ----------------
            # one DMA; 13 tiny matmuls fold everything into a [1, 76] row:
            #   v_row[0:68]=v_c  v_row[68]=const  v_row[69:76]=v_e
            # then ones x row replicates it across the 128 partitions.
            wta_sb = sb.tile([P, 2, NW + 3], F32, tag="wta")
            wtb_sb = sb.tile([P, 2, NW + 3], F32, tag="wtb")
            nc.sync.dma_start(out=wta_sb[:], in_=wta_d[:].rearrange("(p c) j -> p c j", p=P))
            nc.sync.dma_start(out=wtb_sb[:], in_=wtb_d[:].rearrange("(p c) j -> p c j", p=P))
            wt_sbs = [wta_sb[:, 0, :], wta_sb[:, 1, :], wtb_sb[:, 0, :], wtb_sb[:, 1, :]]

            # col 68 accumulates the whole constant: bc.Ws_c (+ be.Ws_e + bs
            # via cross-group accumulation below), so the body needs a single
            # scalar add for ctx's constant term
            v_ps = ps.tile([1, NW], F32, tag="vps")
            for c in range(KC):
                nc.tensor.matmul(
                    out=v_ps[0:1, 0:69],
                    lhsT=wt_sbs[c][:, NW:NW + 1], rhs=wt_sbs[c][:, 0:69],
                    start=(c == 0), stop=False, skip_group_check=True,
                )
            for c in range(KC):
                nc.tensor.matmul(
                    out=v_ps[0:1, 68:69],
                    lhsT=wt_sbs[c][:, NW + 1:NW + 2], rhs=wt_sbs[c][:, 76:77],
                    start=False, stop=False, skip_group_check=True,
                )
            # bs row is nonzero only at h=0 (partition 0, chunk 0)
            nc.tensor.matmul(
                out=v_ps[0:1, 68:69],
                lhsT=wt_sbs[0][:, NW + 2:NW + 3], rhs=wt_sbs[0][:, 77:78],
                start=False, stop=True, skip_group_check=True,
            )
            for c in range(KC):
                nc.tensor.matmul(
                    out=v_ps[0:1, 69:76],
                    lhsT=wt_sbs[c][:, NW + 1:NW + 2], rhs=wt_sbs[c][:, 69:76],
                    start=(c == 0), stop=(c == KC - 1),
                )
            v_row = sb.tile([1, 76], F32, tag="vrow")
            nc.vector.tensor_copy(out=v_row[:], in_=v_ps[0:1, 0:76])
            ones1 = sb.tile([1, P], F32, tag="ones1")
            nc.vector.memset(ones1[:], 1.0)
            vb_ps = ps.tile([P, 76], F32, tag="vbps")
            nc.tensor.matmul(out=vb_ps[:], lhsT=ones1[:], rhs=v_row[:], start=True, stop=True)
            vb = vb_ps  # DVE reads the broadcast directly from PSUM

            # ---------------- main data-parallel body ----------------
            # inputs, grouped [128, G, cols]: row b = g*128 + p
            xin_sb = sb.tile([P, G, NF * E], F32, tag="xin")
            xc_sb = sb.tile([P, G, 68], F32, tag="xc")
            # xc first: ctx (the body's first DVE op) waits on it; xin isn't
            # read until the MAC chain ~1.5us later
            nc.sync.dma_start(out=xc_sb[:], in_=xc_d[:].rearrange("(p g) f -> p g f", p=P))
            nc.sync.dma_start(out=xin_sb[:], in_=xin_d[:].rearrange("(p g) f -> p g f", p=P))

            # ctx[b] = x_ctx[b].v_c + bc.Ws_c + be.Ws_e + bs
            ctx_scr = sb.tile([P, G, 68], F32, tag="ctxscr")
            nc.vector.tensor_tensor(
                out=ctx_scr[:], in0=xc_sb[:],
                in1=_bview(vb[:, 0:68], [(0, G), (1, 68)]), op=ALU.mult,
            )
            ctx_sb = sb.tile([P, G], F32, tag="ctx")
            nc.vector.tensor_reduce(out=ctx_sb[:], in_=ctx_scr[:], axis=mybir.AxisListType.X, op=ALU.add)
            nc.vector.tensor_scalar_add(ctx_sb[:], ctx_sb[:], vb[:, 68:69])

            # expert logits: MAC chain over the 7 features (all unit-stride);
            # the first op seeds with ctx
            eg_sb = sb.tile([P, G, 2 * E], F32, tag="eg")
            acc = eg_sb[:, :, 0:E]

            def xslice(i):
                xv = _bview(xin_sb[:], [(NF * E, G), (1, E)])
                return AP(xv.tensor, xv.offset + i * E, xv.ap)

            # all 7 features on DVE: GPSIMD shares SBUF ports with DVE, so
            # offloading there slows every overlapped DVE op ~2x (measured)
            for i in range(NF):
                seed = _bview(ctx_sb[:], [(1, G), (0, E)]) if i == 0 else acc
                nc.vector.scalar_tensor_tensor(
                    out=acc, in0=xslice(i), scalar=vb[:, 69 + i:70 + i], in1=seed,
                    op0=ALU.mult, op1=ALU.add,
                )

            # top-8 (DVE) runs while exp (ACT) computes; |logits| < 3 so raw exp
            # is safe and softmax(top8) = exp*mask / sum(exp*mask)
            m8 = sb.tile([P, G, 8], F32, tag="m8")
            for g in range(G):
                nc.vector.max(out=m8[:, g, :], in_=eg_sb[:, g, 0:E])
            zb = sb.tile([P, 1], F32, tag="zb")
            nc.vector.memset(zb[:], 0.0)
            ex_sb = sb.tile([P, G, E], F32, tag="ex")
            nc.scalar.activation(out=ex_sb[:], in_=acc, func=ACTF.Exp, bias=zb[:])

            gu_sb = sb.tile([P, G, E], F32, tag="gu")
            z_sb = sb.tile([P, G], F32, tag="z")
            for g in range(G):
                # gu = (logit >= T) * exp(logit);  z = sum_e gu
                nc.vector.scalar_tensor_tensor(
                    out=gu_sb[:, g, :], in0=eg_sb[:, g, 0:E], scalar=m8[:, g, 7:8],
                    in1=ex_sb[:, g, :], op0=ALU.is_ge, op1=ALU.mult,
                    accum_out=z_sb[:, g:g + 1],
                )
            zi_sb = sb.tile([P, G], F32, tag="zi")
            nc.vector.reciprocal(out=zi_sb[:], in_=z_sb[:])
            gates_sb = eg_sb[:, :, E:2 * E]
            # on DVE (not ACT) so the combined output DMA has a single producer
            nc.vector.tensor_tensor(
                out=gates_sb, in0=gu_sb[:],
                in1=_bview(zi_sb[:], [(1, G), (0, E)]), op=ALU.mult,
            )
            # big output leaves early; the action math below overlaps it
            nc.sync.dma_start(out=ego_d[:].rearrange("(p g) c -> p g c", p=P), in_=eg_sb[:])

            # action[b,a] = sum_e gates[b,e] * xq[b,e,a] (q rows are f=0..2)
            atmp = sb.tile([P, G, FQ, E], F32, tag="atmp")
            eg_base = eg_sb[:]
            gates_b = AP(eg_base.tensor, eg_base.offset + E,
                         [list(eg_base.ap[0]), [2 * E, G], [0, FQ], [1, E]])
            xq_b = _bview(xin_sb[:], [(NF * E, G), (E, FQ), (1, E)])
            nc.vector.tensor_tensor(out=atmp[:], in0=gates_b, in1=xq_b, op=ALU.mult)
            act_sb = sb.tile([P, G, FQ + 1], F32, tag="act")
            nc.vector.memset(act_sb[:, :, FQ:FQ + 1], 0.0)
            nc.vector.tensor_reduce(
                out=act_sb[:, :, 0:FQ], in_=atmp[:],
                axis=mybir.AxisListType.X, op=ALU.add,
            )
            nc.sync.dma_start(out=act_d[:].rearrange("(p g) c -> p g c", p=P), in_=act_sb[:])

            # wait-carrier templates for _split_waits: harmless 1-elem ops,
            # one private scratch tile per engine (no cross-engine deps)
            wscr_v = sb.tile([1, 1], F32, tag="wscr_v")
            wscr_g = sb.tile([1, 1], F32, tag="wscr_g")
            wscr_a = sb.tile([1, 1], F32, tag="wscr_a")
            tmpl_v = nc.vector.memset(wscr_v[:], 0.0)
            tmpl_g = nc.gpsimd.memset(wscr_g[:], 0.0)
            tmpl_a = nc.scalar.mul(out=wscr_a[:], in_=wscr_a[:], mul=0.0)

    nc._carrier_templates = {
        mybir.EngineType.DVE: tmpl_v.ins,
        mybir.EngineType.Pool: tmpl_g.ins,
        mybir.EngineType.Activation: tmpl_a.ins,
    }
    return nc


_PROGRAM: bass.Bass | None = None


def _program() -> bass.Bass:
    """Program for hardware execution (waits split for walrus codegen)."""
    global _PROGRAM
    if _PROGRAM is None:
        _PROGRAM = _build_program()
        _strip_tail(_PROGRAM)
        _split_waits(_PROGRAM)
    return _PROGRAM


def _marshal(inputs: dict[str, np.ndarray]):
    f = lambda a: np.asarray(a, dtype=np.float32)
    # feature-major expert inputs: [b, f, e]
    xin = np.concatenate([
        f(inputs["x_q_values"]).transpose(0, 2, 1),   # [B, 3, E]
        f(inputs["x_reward"]).transpose(0, 2, 1),     # [B, 2, E]
        f(inputs["x_risk"]).transpose(0, 2, 1),       # [B, 2, E]
    ], axis=1).reshape(B, NF * E)
    xc = f(inputs["x_context"])
    bs_row = np.zeros((1, H), np.float32)
    bs_row[0, 0] = np.float32(inputs["bs"])
    wx = np.concatenate([
        f(inputs["Wc"]),
        f(inputs["bc"]).reshape(1, H),
        f(inputs["We"]),
        f(inputs["be"]).reshape(1, H),
        bs_row,
    ], axis=0)                                        # [78, H]
    ws = f(inputs["Ws"]).reshape(2 * H)
    e0 = np.zeros(H, np.float32)
    e0[0] = 1.0
    wt = np.concatenate([wx.T, ws[H:, None], ws[:H, None], e0[:, None]], axis=1)
    return xin, xc, np.ascontiguousarray(wt)


def _pmajor(a: np.ndarray) -> np.ndarray:
    """reorder [G*P, cols] rows from g-major (logical) to p-major (DMA)."""
    return np.ascontiguousarray(
        a.reshape(G, P, a.shape[1]).transpose(1, 0, 2).reshape(a.shape))


def _in_maps(inputs: dict[str, np.ndarray]) -> list[dict[str, np.ndarray]]:
    xin, xc, wt = _marshal(inputs)
    # per-half p-major: half h rows r = p*2 + c over its two chunks
    wt_pm = [
        np.ascontiguousarray(
            wt[h * H // 2:(h + 1) * H // 2]
            .reshape(2, P, wt.shape[1]).transpose(1, 0, 2).reshape(H // 2, wt.shape[1]))
        for h in range(2)
    ]
    maps = []
    for i in range(NCORES):
        sl = slice(i * BS, (i + 1) * BS)
        maps.append({
            "xin": _pmajor(np.ascontiguousarray(xin[sl])),
            "xc": _pmajor(np.ascontiguousarray(xc[sl])),
            "WTA": wt_pm[0], "WTB": wt_pm[1],
        })
    return maps


def kernel(**inputs: np.ndarray):
    nc = _program()
    res = run_bass_kernel_spmd(nc, _in_maps(inputs), list(range(NCORES))).results
    # undo the partition-major row order per core shard
    unpm = lambda a: a.reshape(P, G, -1).transpose(1, 0, 2).reshape(BS, -1)
    ego = np.concatenate([unpm(res[i]["ego"]) for i in range(NCORES)], axis=0)
    acto = np.concatenate([unpm(res[i]["acto"]) for i in range(NCORES)], axis=0)
    return (
        np.ascontiguousarray(acto[:, 0:FQ]),
        np.ascontiguousarray(ego[:, E:2 * E]),
        np.ascontiguousarray(ego[:, :E]),
    )


# revision 59
# speedup vs baseline: 1.0363x; 1.0363x over previous
"""MoE router kernel for Trainium2 (Bass/Tile), 8-core data-parallel SPMD.

Per row b (B=4096 total, 512 per core, laid out as 128 partitions x 4 groups):
  expert_logits[b,e] = sum_f x[b,e,f]*v_e[f] + x_ctx[b].v_c + const   (E=64, f=7)
  gate_weights[b,:]  = softmax over top-8 of expert_logits (others 0)
  action_logits[b,a] = sum_e gate[b,e] * x_q[b,e,a]                    (a=3)
with v_e = We @ Ws[:H], v_c = Wc @ Ws[H:], const = bc.Ws[H:] + be.Ws[:H] + bs
folded on device by tiny PE matmuls over host-transposed weights, then
replicated across partitions with a ones-matmul. (Associativity fold
validated: top-8 sets match the reference exactly on the fixed seed inputs,
and logits are bounded |l|<3 so exp() without max-subtraction is safe in f32.)

Inputs are host-marshalled (pure layout, no host math): per-expert features
reordered feature-major so all DVE reads are unit-stride, weights stacked and
transposed into one [512, 81] matrix = [ (Wc|bc|We|be|bs-row)^T | Ws_c | Ws_e | e0 ].
"""

import sys

for _p in ("/opt/trn_rl_repo", "/root/.axon_site/_ro/trn_rl_repo"):
    if _p not in sys.path:
        sys.path.append(_p)

import numpy as np

import concourse.bass as bass
import concourse.mybir as mybir
from concourse.bass_types import AP
from concourse.bass_utils import run_bass_kernel_spmd
from concourse.tile import TileContext

F32 = mybir.dt.float32
ALU = mybir.AluOpType
ACTF = mybir.ActivationFunctionType

B, E, H, TOPK = 4096, 64, 512, 8
NCORES = 8
BS = B // NCORES          # rows per core = 512
P = 128                   # partitions
G = BS // P               # row groups per core = 4
FQ, FR, FK = 3, 2, 2      # per-expert feature widths
NF = FQ + FR + FK         # 7
NW = 68 + 1 + NF + 1 + 1  # stacked rows: Wc | bc | We | be | bs-row = 78
KC = H // P               # 4 contraction chunks


def _bview(ap: AP, free_dims):
    """View `ap` with explicit free-dim (step, count) pairs (partition dim kept)."""
    return AP(ap.tensor, ap.offset, [list(ap.ap[0])] + [list(d) for d in free_dims])


def _split_waits(nc: bass.Bass) -> None:
    """walrus codegen in this environment supports a single sync-wait slot per
    instruction; peel extra waits onto cloned harmless same-engine ops placed
    just before (Drains are peeled onto extra Drains)."""
    import copy

    templates = nc._carrier_templates
    n = 0
    for f in nc.m.functions:
        for blk in f.blocks:
            out = []
            for ins in blk.instructions:
                si = ins.sync_info
                tmpl = templates.get(getattr(ins, "engine", None))
                if type(ins).__name__ == "InstISA" and tmpl is not None:
                    # walrus rejects the barrier's engine-nop ISA op; swap in a
                    # harmless same-engine memset with identical sync_info
                    car = copy.deepcopy(tmpl)
                    car.name = ins.name
                    car.sync_info = si
                    out.append(car)
                    continue
                is_drain = type(ins).__name__ == "InstDrain"
                if is_drain and si is not None and si.on_wait:
                    # the all-engine barrier that follows already guarantees
                    # engine completion; the drain only needs DMA-queue quiesce
                    dma_waits = [w for w in si.on_wait if (w.ant_name or "").startswith("DMA")]
                    if dma_waits:
                        si.on_wait = dma_waits
                        ins.sync_info = si
                if si is not None and si.on_wait and len(si.on_wait) > 1 and (tmpl is not None or is_drain):
                    waits = list(si.on_wait)
                    for w in waits[:-1]:
                        if is_drain:
                            car = mybir.InstDrain(name=f"waitcar-{n}", ins=[], outs=[])
                            car.engine = ins.engine
                        else:
                            car = copy.deepcopy(tmpl)
                            car.name = f"waitcar-{n}"
                        n += 1
                        car.sync_info = mybir.SyncInfo(on_wait=[w], on_update=[])
                        out.append(car)
                    si.on_wait = [waits[-1]]
                    ins.sync_info = si
                out.append(ins)
            blk.instructions = out


def _strip_tail(nc: bass.Bass) -> None:
    """Drop the startup all-engine barrier and the end-of-kernel barriers +
    per-semaphore clears. They exist so a loaded NEFF can be re-executed;
    every kernel() call here builds a fresh executable (semaphores are reset
    at NEFF load), and nothing in this kernel depends on the prologue const-AP
    memsets (Exp uses an explicit zero-bias tile). Only the DMA-quiesce drain
    (output data reaching DRAM) must stay."""
    for f in nc.m.functions:
        for blk in f.blocks:
            insts = list(blk.instructions)
            if not insts:
                continue
            if blk.name == "main":
                blk.instructions = [
                    i for i in insts
                    if type(i).__name__ not in ("InstDrain", "InstEventSemaphore")
                ]
                continue
            first = insts[0]
            if type(first).__name__ != "InstDrain":
                continue
            si = first.sync_info
            if not (si and si.on_wait and any((w.ant_name or "").startswith("DMA") for w in si.on_wait)):
                continue
            kept = [first]
            for ins in insts[1:]:
                if type(ins).__name__ in ("InstEventSemaphore", "InstDrain", "InstISA"):
                    continue
                kept.append(ins)
            blk.instructions = kept


def _build_program() -> bass.Bass:
    nc = bass.Bass()

    # per-expert features, feature-major: [b, f*64+e]; f: q0 q1 q2 r0 r1 k0 k1
    xin_d = nc.declare_dram_parameter("xin", [BS, NF * E], F32, isOutput=False)
    xc_d = nc.declare_dram_parameter("xc", [BS, 68], F32, isOutput=False)
    # [ WX^T (78 cols) | Ws_c | Ws_e | e0 ]; WX rows = Wc|bc|We|be|bs-row
    wt_d = nc.declare_dram_parameter("WT", [H, NW + 3], F32, isOutput=False)

    # outputs: [expert_logits(64) | gates(64)] per row, and [action(3) | pad]
    ego_d = nc.declare_dram_parameter("ego", [BS, 2 * E], F32, isOutput=True)
    act_d = nc.declare_dram_parameter("acto", [BS, FQ + 1], F32, isOutput=True)

    with TileContext(nc) as tc:
        with (
            tc.tile_pool(name="sb", bufs=1) as sb,
            tc.tile_pool(name="ps", bufs=1, space="PSUM") as ps,
        ):
            # ---------------- weight folding on PE ----------------
            # one DMA; 13 tiny matmuls fold everything into a [1, 76] row:
            #   v_row[0:68]=v_c  v_row[68]=const  v_row[69:76]=v_e
            # then ones x row replicates it across the 128 partitions.
            wt_sb = sb.tile([P, KC, NW + 3], F32, tag="wt")
            nc.sync.dma_start(out=wt_sb[:], in_=wt_d[:].rearrange("(p c) j -> p c j", p=P))
            wt_sbs = [wt_sb[:, c, :] for c in range(KC)]

            # col 68 accumulates the whole constant: bc.Ws_c (+ be.Ws_e + bs
            # via cross-group accumulation below), so the body needs a single
            # scalar add for ctx's constant term
            v_ps = ps.tile([1, NW], F32, tag="vps")
            for c in range(KC):
                nc.tensor.matmul(
                    out=v_ps[0:1, 0:69],
                    lhsT=wt_sbs[c][:, NW:NW + 1], rhs=wt_sbs[c][:, 0:69],
                    start=(c == 0), stop=False, skip_group_check=True,
                )
            for c in range(KC):
                nc.tensor.matmul(
                    out=v_ps[0:1, 68:69],
                    lhsT=wt_sbs[c][:, NW + 1:NW + 2], rhs=wt_sbs[c][:, 76:77],
                    start=False, stop=False, skip_group_check=True,
                )
            # bs row is nonzero only at h=0 (partition 0, chunk 0)
            nc.tensor.matmul(
                out=v_ps[0:1, 68:69],
                lhsT=wt_sbs[0][:, NW + 2:NW + 3], rhs=wt_sbs[0][:, 77:78],
                start=False, stop=True, skip_group_check=True,
            )
            for c in range(KC):
                nc.tensor.matmul(
                    out=v_ps[0:1, 69:76],
                    lhsT=wt_sbs[c][:, NW + 1:NW + 2], rhs=wt_sbs[c][:, 69:76],
                    start=(c == 0), stop=(c == KC - 1),
                )
            v_row = sb.tile([1, 76], F32, tag="vrow")
            nc.vector.tensor_copy(out=v_row[:], in_=v_ps[0:1, 0:76])
            ones1 = sb.tile([1, P], F32, tag="ones1")
            nc.vector.memset(ones1[:], 1.0)
            vb_ps = ps.tile([P, 76], F32, tag="vbps")
            nc.tensor.matmul(out=vb_ps[:], lhsT=ones1[:], rhs=v_row[:], start=True, stop=True)
            vb = vb_ps  # DVE reads the broadcast directly from PSUM

            # ---------------- main data-parallel body ----------------
            # inputs, grouped [128, G, cols]: row b = g*128 + p
            xin_sb = sb.tile([P, G, NF * E], F32, tag="xin")
            xc_sb = sb.tile([P, G, 68], F32, tag="xc")
            # xc first: ctx (the body's first DVE op) waits on it; xin isn't
            # read until the MAC chain ~1.5us later
            nc.sync.dma_start(out=xc_sb[:], in_=xc_d[:].rearrange("(p g) f -> p g f", p=P))
            nc.sync.dma_start(out=xin_sb[:], in_=xin_d[:].rearrange("(p g) f -> p g f", p=P))

            # ctx[b] = x_ctx[b].v_c + bc.Ws_c + be.Ws_e + bs
            ctx_scr = sb.tile([P, G, 68], F32, tag="ctxscr")
            nc.vector.tensor_tensor(
                out=ctx_scr[:], in0=xc_sb[:],
                in1=_bview(vb[:, 0:68], [(0, G), (1, 68)]), op=ALU.mult,
            )
            ctx_sb = sb.tile([P, G], F32, tag="ctx")
            nc.vector.tensor_reduce(out=ctx_sb[:], in_=ctx_scr[:], axis=mybir.AxisListType.X, op=ALU.add)
            nc.vector.tensor_scalar_add(ctx_sb[:], ctx_sb[:], vb[:, 68:69])

            # expert logits: MAC chain over the 7 features (all unit-stride);
            # the first op seeds with ctx
            eg_sb = sb.tile([P, G, 2 * E], F32, tag="eg")
            acc = eg_sb[:, :, 0:E]

            def xslice(i):
                xv = _bview(xin_sb[:], [(NF * E, G), (1, E)])
                return AP(xv.tensor, xv.offset + i * E, xv.ap)

            # all 7 features on DVE: GPSIMD shares SBUF ports with DVE, so
            # offloading there slows every overlapped DVE op ~2x (measured)
            for i in range(NF):
                seed = _bview(ctx_sb[:], [(1, G), (0, E)]) if i == 0 else acc
                nc.vector.scalar_tensor_tensor(
                    out=acc, in0=xslice(i), scalar=vb[:, 69 + i:70 + i], in1=seed,
                    op0=ALU.mult, op1=ALU.add,
                )

            # top-8 (DVE) runs while exp (ACT) computes; |logits| < 3 so raw exp
            # is safe and softmax(top8) = exp*mask / sum(exp*mask)
            m8 = sb.tile([P, G, 8], F32, tag="m8")
            for g in range(G):
                nc.vector.max(out=m8[:, g, :], in_=eg_sb[:, g, 0:E])
            zb = sb.tile([P, 1], F32, tag="zb")
            nc.vector.memset(zb[:], 0.0)
            ex_sb = sb.tile([P, G, E], F32, tag="ex")
            nc.scalar.activation(out=ex_sb[:], in_=acc, func=ACTF.Exp, bias=zb[:])

            gu_sb = sb.tile([P, G, E], F32, tag="gu")
            z_sb = sb.tile([P, G], F32, tag="z")
            for g in range(G):
                # gu = (logit >= T) * exp(logit);  z = sum_e gu
                nc.vector.scalar_tensor_tensor(
                    out=gu_sb[:, g, :], in0=eg_sb[:, g, 0:E], scalar=m8[:, g, 7:8],
                    in1=ex_sb[:, g, :], op0=ALU.is_ge, op1=ALU.mult,
                    accum_out=z_sb[:, g:g + 1],
                )
            zi_sb = sb.tile([P, G], F32, tag="zi")
            nc.vector.reciprocal(out=zi_sb[:], in_=z_sb[:])
            gates_sb = eg_sb[:, :, E:2 * E]
            # on DVE (not ACT) so the combined output DMA has a single producer
            nc.vector.tensor_tensor(
                out=gates_sb, in0=gu_sb[:],
                in1=_bview(zi_sb[:], [(1, G), (0, E)]), op=ALU.mult,
            )
            # big output leaves early; the action math below overlaps it
            nc.sync.dma_start(out=ego_d[:].rearrange("(p g) c -> p g c", p=P), in_=eg_sb[:])

            # action[b,a] = sum_e gates[b,e] * xq[b,e,a] (q rows are f=0..2)
            atmp = sb.tile([P, G, FQ, E], F32, tag="atmp")
            eg_base = eg_sb[:]
            gates_b = AP(eg_base.tensor, eg_base.offset + E,
                         [list(eg_base.ap[0]), [2 * E, G], [0, FQ], [1, E]])
            xq_b = _bview(xin_sb[:], [(NF * E, G), (E, FQ), (1, E)])
            nc.vector.tensor_tensor(out=atmp[:], in0=gates_b, in1=xq_b, op=ALU.mult)
            act_sb = sb.tile([P, G, FQ + 1], F32, tag="act")
            nc.vector.memset(act_sb[:, :, FQ:FQ + 1], 0.0)
            nc.vector.tensor_reduce(
                out=act_sb[:, :, 0:FQ], in_=atmp[:],
                axis=mybir.AxisListType.X, op=ALU.add,
            )
            nc.sync.dma_start(out=act_d[:].rearrange("(p g) c -> p g c", p=P), in_=act_sb[:])

            # wait-carrier templates for _split_waits: harmless 1-elem ops,
            # one private scratch tile per engine (no cross-engine deps)
            wscr_v = sb.tile([1, 1], F32, tag="wscr_v")
            wscr_g = sb.tile([1, 1], F32, tag="wscr_g")
            wscr_a = sb.tile([1, 1], F32, tag="wscr_a")
            tmpl_v = nc.vector.memset(wscr_v[:], 0.0)
            tmpl_g = nc.gpsimd.memset(wscr_g[:], 0.0)
            tmpl_a = nc.scalar.mul(out=wscr_a[:], in_=wscr_a[:], mul=0.0)

    nc._carrier_templates = {
        mybir.EngineType.DVE: tmpl_v.ins,
        mybir.EngineType.Pool: tmpl_g.ins,
        mybir.EngineType.Activation: tmpl_a.ins,
    }
    return nc


_PROGRAM: bass.Bass | None = None


def _program() -> bass.Bass:
    """Program for hardware execution (waits split for walrus codegen)."""
    global _PROGRAM
    if _PROGRAM is None:
        _PROGRAM = _build_program()
        _strip_tail(_PROGRAM)
        _split_waits(_PROGRAM)
    return _PROGRAM


def _marshal(inputs: dict[str, np.ndarray]):
    f = lambda a: np.asarray(a, dtype=np.float32)
    # feature-major expert inputs: [b, f, e]
    xin = np.concatenate([
        f(inputs["x_q_values"]).transpose(0, 2, 1),   # [B, 3, E]
        f(inputs["x_reward"]).transpose(0, 2, 1),     # [B, 2, E]
        f(inputs["x_risk"]).transpose(0, 2, 1),       # [B, 2, E]
    ], axis=1).reshape(B, NF * E)
    xc = f(inputs["x_context"])
    bs_row = np.zeros((1, H), np.float32)
    bs_row[0, 0] = np.float32(inputs["bs"])
    wx = np.concatenate([
        f(inputs["Wc"]),
        f(inputs["bc"]).reshape(1, H),
        f(inputs["We"]),
        f(inputs["be"]).reshape(1, H),
        bs_row,
    ], axis=0)                                        # [78, H]
    ws = f(inputs["Ws"]).reshape(2 * H)
    e0 = np.zeros(H, np.float32)
    e0[0] = 1.0
    wt = np.concatenate([wx.T, ws[H:, None], ws[:H, None], e0[:, None]], axis=1)
    return xin, xc, np.ascontiguousarray(wt)


def _pmajor(a: np.ndarray) -> np.ndarray:
    """reorder [G*P, cols] rows from g-major (logical) to p-major (DMA)."""
    return np.ascontiguousarray(
        a.reshape(G, P, a.shape[1]).transpose(1, 0, 2).reshape(a.shape))


def _in_maps(inputs: dict[str, np.ndarray]) -> list[dict[str, np.ndarray]]:
    xin, xc, wt = _marshal(inputs)
    wt_pm = np.ascontiguousarray(
        wt.reshape(KC, P, wt.shape[1]).transpose(1, 0, 2).reshape(wt.shape))
    maps = []
    for i in range(NCORES):
        sl = slice(i * BS, (i + 1) * BS)
        maps.append({
            "xin": _pmajor(np.ascontiguousarray(xin[sl])),
            "xc": _pmajor(np.ascontiguousarray(xc[sl])),
            "WT": wt_pm,
        })
    return maps


def kernel(**inputs: np.ndarray):
    nc = _program()
    res = run_bass_kernel_spmd(nc, _in_maps(inputs), list(range(NCORES))).results
    # undo the partition-major row order per core shard
    unpm = lambda a: a.reshape(P, G, -1).transpose(1, 0, 2).reshape(BS, -1)
    ego = np.concatenate([unpm(res[i]["ego"]) for i in range(NCORES)], axis=0)
    acto = np.concatenate([unpm(res[i]["acto"]) for i in range(NCORES)], axis=0)
    return (
        np.ascontiguousarray(acto[:, 0:FQ]),
        np.ascontiguousarray(ego[:, E:2 * E]),
        np.ascontiguousarray(ego[:, :E]),
    )


# revision 60
# speedup vs baseline: 1.0907x; 1.0525x over previous
"""MoE router kernel for Trainium2 (Bass/Tile), 8-core data-parallel SPMD.

Per row b (B=4096 total, 512 per core, laid out as 128 partitions x 4 groups):
  expert_logits[b,e] = sum_f x[b,e,f]*v_e[f] + x_ctx[b].v_c + const   (E=64, f=7)
  gate_weights[b,:]  = softmax over top-8 of expert_logits (others 0)
  action_logits[b,a] = sum_e gate[b,e] * x_q[b,e,a]                    (a=3)
with v_e = We @ Ws[:H], v_c = Wc @ Ws[H:], const = bc.Ws[H:] + be.Ws[:H] + bs
folded on device by tiny PE matmuls over host-transposed weights, then
replicated across partitions with a ones-matmul. (Associativity fold
validated: top-8 sets match the reference exactly on the fixed seed inputs,
and logits are bounded |l|<3 so exp() without max-subtraction is safe in f32.)

Inputs are host-marshalled (pure layout, no host math): per-expert features
reordered feature-major so all DVE reads are unit-stride, weights stacked and
transposed into one [512, 81] matrix = [ (Wc|bc|We|be|bs-row)^T | Ws_c | Ws_e | e0 ].
"""

import sys

for _p in ("/opt/trn_rl_repo", "/root/.axon_site/_ro/trn_rl_repo"):
    if _p not in sys.path:
        sys.path.append(_p)

import numpy as np

import concourse.bass as bass
import concourse.mybir as mybir
from concourse.bass_types import AP
from concourse.bass_utils import run_bass_kernel_spmd
from concourse.tile import TileContext

F32 = mybir.dt.float32
ALU = mybir.AluOpType
ACTF = mybir.ActivationFunctionType

B, E, H, TOPK = 4096, 64, 512, 8
NCORES = 8
BS = B // NCORES          # rows per core = 512
P = 128                   # partitions
G = BS // P               # row groups per core = 4
FQ, FR, FK = 3, 2, 2      # per-expert feature widths
NF = FQ + FR + FK         # 7
NW = 68 + 1 + NF + 1 + 1  # stacked rows: Wc | bc | We | be | bs-row = 78
KC = H // P               # 4 contraction chunks


def _bview(ap: AP, free_dims):
    """View `ap` with explicit free-dim (step, count) pairs (partition dim kept)."""
    return AP(ap.tensor, ap.offset, [list(ap.ap[0])] + [list(d) for d in free_dims])


def _split_waits(nc: bass.Bass) -> None:
    """walrus codegen in this environment supports a single sync-wait slot per
    instruction; peel extra waits onto cloned harmless same-engine ops placed
    just before (Drains are peeled onto extra Drains)."""
    import copy

    templates = nc._carrier_templates
    n = 0
    for f in nc.m.functions:
        for blk in f.blocks:
            out = []
            for ins in blk.instructions:
                si = ins.sync_info
                tmpl = templates.get(getattr(ins, "engine", None))
                if type(ins).__name__ == "InstISA" and tmpl is not None:
                    # walrus rejects the barrier's engine-nop ISA op; swap in a
                    # harmless same-engine memset with identical sync_info
                    car = copy.deepcopy(tmpl)
                    car.name = ins.name
                    car.sync_info = si
                    out.append(car)
                    continue
                is_drain = type(ins).__name__ == "InstDrain"
                if is_drain and si is not None and si.on_wait:
                    # the all-engine barrier that follows already guarantees
                    # engine completion; the drain only needs DMA-queue quiesce
                    dma_waits = [w for w in si.on_wait if (w.ant_name or "").startswith("DMA")]
                    if dma_waits:
                        si.on_wait = dma_waits
                        ins.sync_info = si
                if si is not None and si.on_wait and len(si.on_wait) > 1 and (tmpl is not None or is_drain):
                    waits = list(si.on_wait)
                    for w in waits[:-1]:
                        if is_drain:
                            car = mybir.InstDrain(name=f"waitcar-{n}", ins=[], outs=[])
                            car.engine = ins.engine
                        else:
                            car = copy.deepcopy(tmpl)
                            car.name = f"waitcar-{n}"
                        n += 1
                        car.sync_info = mybir.SyncInfo(on_wait=[w], on_update=[])
                        out.append(car)
                    si.on_wait = [waits[-1]]
                    ins.sync_info = si
                out.append(ins)
            blk.instructions = out


def _strip_tail(nc: bass.Bass) -> None:
    """Drop the startup all-engine barrier and the end-of-kernel barriers +
    per-semaphore clears. They exist so a loaded NEFF can be re-executed;
    every kernel() call here builds a fresh executable (semaphores are reset
    at NEFF load), and nothing in this kernel depends on the prologue const-AP
    memsets (Exp uses an explicit zero-bias tile). Only the DMA-quiesce drain
    (output data reaching DRAM) must stay."""
    for f in nc.m.functions:
        for blk in f.blocks:
            insts = list(blk.instructions)
            if not insts:
                continue
            if blk.name == "main":
                blk.instructions = [
                    i for i in insts
                    if type(i).__name__ not in ("InstDrain", "InstEventSemaphore")
                ]
                continue
            first = insts[0]
            if type(first).__name__ != "InstDrain":
                continue
            si = first.sync_info
            if not (si and si.on_wait and any((w.ant_name or "").startswith("DMA") for w in si.on_wait)):
                continue
            kept = [first]
            for ins in insts[1:]:
                if type(ins).__name__ in ("InstEventSemaphore", "InstDrain", "InstISA"):
                    continue
                kept.append(ins)
            blk.instructions = kept


def _build_program() -> bass.Bass:
    nc = bass.Bass()

    # per-expert features, feature-major: [b, f*64+e]; f: q0 q1 q2 r0 r1 k0 k1
    xin_d = nc.declare_dram_parameter("xin", [BS, NF * E], F32, isOutput=False)
    xc_d = nc.declare_dram_parameter("xc", [BS, 68], F32, isOutput=False)
    # [ WX^T (78 cols) | Ws_c | Ws_e | e0 ]; WX rows = Wc|bc|We|be|bs-row
    wt_d = nc.declare_dram_parameter("WT", [H, NW + 3], F32, isOutput=False)

    # three single-producer outputs: logits (DVE), gates (ACT), action (DVE)
    elog_d = nc.declare_dram_parameter("elog", [BS, E], F32, isOutput=True)
    gates_d = nc.declare_dram_parameter("gates", [BS, E], F32, isOutput=True)
    act_d = nc.declare_dram_parameter("acto", [BS, FQ + 1], F32, isOutput=True)

    with TileContext(nc) as tc:
        with (
            tc.tile_pool(name="sb", bufs=1) as sb,
            tc.tile_pool(name="ps", bufs=1, space="PSUM") as ps,
        ):
            # ---------------- weight folding on PE ----------------
            # one DMA; 13 tiny matmuls fold everything into a [1, 76] row:
            #   v_row[0:68]=v_c  v_row[68]=const  v_row[69:76]=v_e
            # then ones x row replicates it across the 128 partitions.
            wt_sb = sb.tile([P, KC, NW + 3], F32, tag="wt")
            nc.sync.dma_start(out=wt_sb[:], in_=wt_d[:].rearrange("(p c) j -> p c j", p=P))
            wt_sbs = [wt_sb[:, c, :] for c in range(KC)]

            # col 68 accumulates the whole constant: bc.Ws_c (+ be.Ws_e + bs
            # via cross-group accumulation below), so the body needs a single
            # scalar add for ctx's constant term
            v_ps = ps.tile([1, NW], F32, tag="vps")
            for c in range(KC):
                nc.tensor.matmul(
                    out=v_ps[0:1, 0:69],
                    lhsT=wt_sbs[c][:, NW:NW + 1], rhs=wt_sbs[c][:, 0:69],
                    start=(c == 0), stop=False, skip_group_check=True,
                )
            for c in range(KC):
                nc.tensor.matmul(
                    out=v_ps[0:1, 68:69],
                    lhsT=wt_sbs[c][:, NW + 1:NW + 2], rhs=wt_sbs[c][:, 76:77],
                    start=False, stop=False, skip_group_check=True,
                )
            # bs row is nonzero only at h=0 (partition 0, chunk 0)
            nc.tensor.matmul(
                out=v_ps[0:1, 68:69],
                lhsT=wt_sbs[0][:, NW + 2:NW + 3], rhs=wt_sbs[0][:, 77:78],
                start=False, stop=True, skip_group_check=True,
            )
            for c in range(KC):
                nc.tensor.matmul(
                    out=v_ps[0:1, 69:76],
                    lhsT=wt_sbs[c][:, NW + 1:NW + 2], rhs=wt_sbs[c][:, 69:76],
                    start=(c == 0), stop=(c == KC - 1),
                )
            v_row = sb.tile([1, 76], F32, tag="vrow")
            nc.vector.tensor_copy(out=v_row[:], in_=v_ps[0:1, 0:76])
            ones1 = sb.tile([1, P], F32, tag="ones1")
            nc.vector.memset(ones1[:], 1.0)
            vb_ps = ps.tile([P, 76], F32, tag="vbps")
            nc.tensor.matmul(out=vb_ps[:], lhsT=ones1[:], rhs=v_row[:], start=True, stop=True)
            vb = vb_ps  # DVE reads the broadcast directly from PSUM

            # ---------------- main data-parallel body ----------------
            # inputs, grouped [128, G, cols]: row b = g*128 + p
            xin_sb = sb.tile([P, G, NF * E], F32, tag="xin")
            xc_sb = sb.tile([P, G, 68], F32, tag="xc")
            # xc first: ctx (the body's first DVE op) waits on it; xin isn't
            # read until the MAC chain ~1.5us later
            nc.sync.dma_start(out=xc_sb[:], in_=xc_d[:].rearrange("(p g) f -> p g f", p=P))
            nc.sync.dma_start(out=xin_sb[:], in_=xin_d[:].rearrange("(p g) f -> p g f", p=P))

            # ctx[b] = x_ctx[b].v_c + bc.Ws_c + be.Ws_e + bs
            ctx_scr = sb.tile([P, G, 68], F32, tag="ctxscr")
            nc.vector.tensor_tensor(
                out=ctx_scr[:], in0=xc_sb[:],
                in1=_bview(vb[:, 0:68], [(0, G), (1, 68)]), op=ALU.mult,
            )
            ctx_sb = sb.tile([P, G], F32, tag="ctx")
            nc.vector.tensor_reduce(out=ctx_sb[:], in_=ctx_scr[:], axis=mybir.AxisListType.X, op=ALU.add)
            nc.vector.tensor_scalar_add(ctx_sb[:], ctx_sb[:], vb[:, 68:69])

            # expert logits: MAC chain over the 7 features (all unit-stride);
            # the first op seeds with ctx
            acc_sb = sb.tile([P, G, E], F32, tag="acc")
            acc = acc_sb[:]

            def xslice(i):
                xv = _bview(xin_sb[:], [(NF * E, G), (1, E)])
                return AP(xv.tensor, xv.offset + i * E, xv.ap)

            # all 7 features on DVE: GPSIMD shares SBUF ports with DVE, so
            # offloading there slows every overlapped DVE op ~2x (measured)
            for i in range(NF):
                seed = _bview(ctx_sb[:], [(1, G), (0, E)]) if i == 0 else acc
                nc.vector.scalar_tensor_tensor(
                    out=acc, in0=xslice(i), scalar=vb[:, 69 + i:70 + i], in1=seed,
                    op0=ALU.mult, op1=ALU.add,
                )

            # top-8 (DVE) runs while exp (ACT) computes; |logits| < 3 so raw exp
            # is safe and softmax(top8) = exp*mask / sum(exp*mask)
            nc.sync.dma_start(out=elog_d[:].rearrange("(p g) e -> p g e", p=P), in_=acc)
            m8 = sb.tile([P, G, 8], F32, tag="m8")
            for g in range(G):
                nc.vector.max(out=m8[:, g, :], in_=acc_sb[:, g, :])
            zb = sb.tile([P, 1], F32, tag="zb")
            nc.vector.memset(zb[:], 0.0)
            ex_sb = sb.tile([P, G, E], F32, tag="ex")
            nc.scalar.activation(out=ex_sb[:], in_=acc, func=ACTF.Exp, bias=zb[:])

            gu_sb = sb.tile([P, G, E], F32, tag="gu")
            z_sb = sb.tile([P, G], F32, tag="z")
            for g in range(G):
                # gu = (logit >= T) * exp(logit);  z = sum_e gu
                nc.vector.scalar_tensor_tensor(
                    out=gu_sb[:, g, :], in0=acc_sb[:, g, :], scalar=m8[:, g, 7:8],
                    in1=ex_sb[:, g, :], op0=ALU.is_ge, op1=ALU.mult,
                    accum_out=z_sb[:, g:g + 1],
                )
            zi_sb = sb.tile([P, G], F32, tag="zi")
            nc.vector.reciprocal(out=zi_sb[:], in_=z_sb[:])
            # normalize on the idle ACT engine while DVE runs the action bmm
            gates_sb = sb.tile([P, G, E], F32, tag="gates")
            for g in range(G):
                nc.scalar.mul(out=gates_sb[:, g, :], in_=gu_sb[:, g, :], mul=zi_sb[:, g:g + 1])
            nc.sync.dma_start(out=gates_d[:].rearrange("(p g) e -> p g e", p=P), in_=gates_sb[:])

            # action from unnormalized gu, scaled by 1/z at the end:
            # act[b,a] = (sum_e gu[b,e] * xq[b,e,a]) / z[b]
            atmp = sb.tile([P, G, FQ, E], F32, tag="atmp")
            gu_b = _bview(gu_sb[:], [(E, G), (0, FQ), (1, E)])
            xq_b = _bview(xin_sb[:], [(NF * E, G), (E, FQ), (1, E)])
            nc.vector.tensor_tensor(out=atmp[:], in0=gu_b, in1=xq_b, op=ALU.mult)
            act_sb = sb.tile([P, G, FQ + 1], F32, tag="act")
            nc.vector.memset(act_sb[:, :, FQ:FQ + 1], 0.0)
            nc.vector.tensor_reduce(
                out=act_sb[:, :, 0:FQ], in_=atmp[:],
                axis=mybir.AxisListType.X, op=ALU.add,
            )
            nc.vector.tensor_tensor(
                out=act_sb[:, :, 0:FQ], in0=act_sb[:, :, 0:FQ],
                in1=_bview(zi_sb[:], [(1, G), (0, FQ)]), op=ALU.mult,
            )
            nc.sync.dma_start(out=act_d[:].rearrange("(p g) c -> p g c", p=P), in_=act_sb[:])

            # wait-carrier templates for _split_waits: harmless 1-elem ops,
            # one private scratch tile per engine (no cross-engine deps)
            wscr_v = sb.tile([1, 1], F32, tag="wscr_v")
            wscr_g = sb.tile([1, 1], F32, tag="wscr_g")
            wscr_a = sb.tile([1, 1], F32, tag="wscr_a")
            tmpl_v = nc.vector.memset(wscr_v[:], 0.0)
            tmpl_g = nc.gpsimd.memset(wscr_g[:], 0.0)
            tmpl_a = nc.scalar.mul(out=wscr_a[:], in_=wscr_a[:], mul=0.0)

    nc._carrier_templates = {
        mybir.EngineType.DVE: tmpl_v.ins,
        mybir.EngineType.Pool: tmpl_g.ins,
        mybir.EngineType.Activation: tmpl_a.ins,
    }
    return nc


_PROGRAM: bass.Bass | None = None


def _program() -> bass.Bass:
    """Program for hardware execution (waits split for walrus codegen)."""
    global _PROGRAM
    if _PROGRAM is None:
        _PROGRAM = _build_program()
        _strip_tail(_PROGRAM)
        _split_waits(_PROGRAM)
    return _PROGRAM


def _marshal(inputs: dict[str, np.ndarray]):
    f = lambda a: np.asarray(a, dtype=np.float32)
    # feature-major expert inputs: [b, f, e]
    xin = np.concatenate([
        f(inputs["x_q_values"]).transpose(0, 2, 1),   # [B, 3, E]
        f(inputs["x_reward"]).transpose(0, 2, 1),     # [B, 2, E]
        f(inputs["x_risk"]).transpose(0, 2, 1),       # [B, 2, E]
    ], axis=1).reshape(B, NF * E)
    xc = f(inputs["x_context"])
    bs_row = np.zeros((1, H), np.float32)
    bs_row[0, 0] = np.float32(inputs["bs"])
    wx = np.concatenate([
        f(inputs["Wc"]),
        f(inputs["bc"]).reshape(1, H),
        f(inputs["We"]),
        f(inputs["be"]).reshape(1, H),
        bs_row,
    ], axis=0)                                        # [78, H]
    ws = f(inputs["Ws"]).reshape(2 * H)
    e0 = np.zeros(H, np.float32)
    e0[0] = 1.0
    wt = np.concatenate([wx.T, ws[H:, None], ws[:H, None], e0[:, None]], axis=1)
    return xin, xc, np.ascontiguousarray(wt)


def _pmajor(a: np.ndarray) -> np.ndarray:
    """reorder [G*P, cols] rows from g-major (logical) to p-major (DMA)."""
    return np.ascontiguousarray(
        a.reshape(G, P, a.shape[1]).transpose(1, 0, 2).reshape(a.shape))


def _in_maps(inputs: dict[str, np.ndarray]) -> list[dict[str, np.ndarray]]:
    xin, xc, wt = _marshal(inputs)
    wt_pm = np.ascontiguousarray(
        wt.reshape(KC, P, wt.shape[1]).transpose(1, 0, 2).reshape(wt.shape))
    maps = []
    for i in range(NCORES):
        sl = slice(i * BS, (i + 1) * BS)
        maps.append({
            "xin": _pmajor(np.ascontiguousarray(xin[sl])),
            "xc": _pmajor(np.ascontiguousarray(xc[sl])),
            "WT": wt_pm,
        })
    return maps


def kernel(**inputs: np.ndarray):
    nc = _program()
    res = run_bass_kernel_spmd(nc, _in_maps(inputs), list(range(NCORES))).results
    # undo the partition-major row order per core shard
    unpm = lambda a: a.reshape(P, G, -1).transpose(1, 0, 2).reshape(BS, -1)
    elog = np.concatenate([unpm(res[i]["elog"]) for i in range(NCORES)], axis=0)
    gates = np.concatenate([unpm(res[i]["gates"]) for i in range(NCORES)], axis=0)
    acto = np.concatenate([unpm(res[i]["acto"]) for i in range(NCORES)], axis=0)
    return (
        np.ascontiguousarray(acto[:, 0:FQ]),
        np.ascontiguousarray(gates),
        np.ascontiguousarray(elog),
    )
